# revision 8
# baseline (speedup 1.0000x reference)
"""DepthGatedModule kernel for 8 Trainium2 NeuronCores (Bass/Tile).

Reference computation (B=4, C=512, H=W=48, N=B*H*W=9216 tokens):
  xt  = tok(x) @ w_rgb.T + b_rgb
  lhs = tok(d) @ w_lhs.T + b_lhs ; rhs = tok(d) @ w_rhs.T + b_rhs
  P   = softmax(lhs @ rhs.T, axis=1) ;  enh = P @ xt
  y   = enh @ w_dec.T + b_dec ;  BatchNorm(train) ; ReLU

Algebraic restructuring (host-side weight folds in float64):
  logits[q,k] = lhs_q . rhs_k = d_q^T (Wl^T Wr) d_k + (Wr^T bl).d_k + f(q)
  The f(q) terms are constant per softmax row and drop out, so with
  M = w_lhs.T @ w_rhs and u = w_rhs.T @ b_lhs the key side of the
  attention needs NO linear at all. Softmax rows sum to 1, so
  P @ (X Wrgb^T + b_rgb) = (P X) Wrgb^T + b_rgb and the rgb linear fuses
  with the decoder: y = (P X)/den @ (w_dec w_rgb).T + (w_dec b_rgb + b_dec).

Device kernel (queries sharded 1152/core, keys streamed in 512-token
chunks): S^T tiles contract the raw fp16 depth chunk (1-pass LDWEIGHTS)
against lhs' = d_q M + u; E = exp(S^T - 40) in bf16 (e^15 range rules
out fp16); PV uses X stationary in bf16 token-major so enhanced comes
out channel-major -- no transposes anywhere. Denominator = one
gpsimd.partition_all_reduce over the DVE-accumulated E sums (no PE
work). The fused decoder linear overlaps the last chunk's PV via
qt-ordered normalize; BatchNorm stats split across DVE (sum y) and ACT
(sum y^2 via accum_out) feed one 4KB AllReduce. PE is warmed with junk
matmuls during the initial DMA so the HAM clock-gate opens before real
work. Matmul issue cadence is ~196ns/tile (framework adds a semaphore
update per matmul); the 18-chunk key loop runs gap-free at ~18.7us.
"""
import numpy as np
import ml_dtypes

import concourse.bacc as bacc
import concourse.bass as bass
import concourse.bass_isa as bass_isa
import concourse.mybir as mybir
import concourse.tile as tile
from concourse.bass_utils import run_bass_kernel_spmd

F32 = mybir.dt.float32
F32R = mybir.dt.float32r
BF16 = mybir.dt.bfloat16
F16 = mybir.dt.float16
AF = mybir.ActivationFunctionType

B, C, H, W = 4, 512, 48, 48
N = B * H * W            # 9216 tokens
NCORES = 8
Q = N // NCORES          # 1152 queries per core
CB = C // 128            # 4 channel blocks
KC_TOK = 512             # key-chunk tokens
NKC = N // KC_TOK        # 18 key chunks
KB = KC_TOK // 128       # 4 key blocks per chunk
QT = 384                 # query tile (free dim of matmuls)
NQT = Q // QT            # 3 query tiles
SHIFT = -40.0            # softmax constant shift (logit max ~55)
BN_EPS = 1e-5

_nc_cache = None


class _SafeBacc(bacc.Bacc):
    """Standalone InstLdweights + non-self-loading matmult silently yields
    all-zero output for float32r on TRN2 hardware. Keep matmul waits on the
    matmul and let generate_event_semaphores() split them into EVSEM chains
    instead of moving them onto an LDWEIGHTS."""

    def move_matmul_waits_to_ldweights(self):
        pass


def _build(nkc=NKC, use_collective=True, stage=4):
    nc = _SafeBacc("TRN2", target_bir_lowering=False, debug=False,
                   num_devices=NCORES)

    d_q = nc.declare_dram_parameter("d_q", [C, Q], F16, isOutput=False)
    d_full = nc.declare_dram_parameter("d_full", [C, N], F16, isOutput=False)
    x_tok = nc.declare_dram_parameter("x_tok", [N, C], BF16, isOutput=False)
    m_t = nc.declare_dram_parameter("M_t", [C, C], F16, isOutput=False)
    w2_t = nc.declare_dram_parameter("W2_t", [C, C], F32, isOutput=False)
    vecs = {
        name: nc.declare_dram_parameter(name, [C], F32, isOutput=False)
        for name in ["u", "b2", "gamma", "beta"]
    }
    y_out = nc.declare_dram_parameter("y", [C, Q], F32, isOutput=True)

    dq_re = d_q.rearrange("(cb p) n -> p cb n", p=128)
    d_re = d_full.rearrange("(cb p) n -> p cb n", p=128)
    x_re = x_tok.rearrange("(nb p) c -> p nb c", p=128)
    y_re = y_out.rearrange("(cb p) n -> p cb n", p=128)

    with tile.TileContext(nc) as tc:
        with (
            tc.tile_pool(name="consts", bufs=1) as consts,
            tc.tile_pool(name="chunks", bufs=3) as chunks,
            tc.tile_pool(name="eall", bufs=2) as eallp,
            tc.tile_pool(name="res", bufs=1) as res,
            tc.tile_pool(name="outp", bufs=4) as outp,
            tc.tile_pool(name="st", bufs=4, space="PSUM") as stp,
            tc.tile_pool(name="pv", bufs=4, space="PSUM") as pvp,
            tc.tile_pool(name="dram", bufs=1, space="DRAM") as dram,
        ):
            # ---- constants / weights ----
            m_sb = consts.tile([128, CB, C], F16, tag="m")
            nc.gpsimd.dma_start(out=m_sb[:, :, :],
                                in_=m_t.rearrange("(cb p) co -> p cb co", p=128))
            v_t = {}
            for name in vecs:
                v = consts.tile([128, CB], F32, tag=f"v_{name}")
                nc.sync.dma_start(out=v[:, :],
                                  in_=vecs[name].rearrange("(cb p) -> p cb", p=128))
                v_t[name] = v

            shift_t = consts.tile([128, 1], F32)
            nc.vector.memset(shift_t, SHIFT)
            ones_st = consts.tile([128, 128], BF16)     # den bcast stationary
            nc.vector.memset(ones_st, 1.0)

            # warm the PE (HAM un-throttle needs ~3.4us of activity) while
            # the first DMAs land; junk matmuls with no data dependencies
            wps = stp.tile([128, 512], F32, tag="st")
            for _ in range(40):
                nc.tensor.matmul(wps[:, :128], lhsT=ones_st[:, :],
                                 rhs=ones_st[:, :], start=True, stop=True)
            # preload the SQRT ACT table off the critical path
            sq_warm = consts.tile([128, 1], F32)
            nc.scalar.sqrt(out=sq_warm[:, :], in_=shift_t[:, :])

            # prefetch key-chunk kc (DMA issue order = program order)
            chunk_tiles = {}

            def issue_chunk(kc):
                k0 = kc * KC_TOK
                d_ch = chunks.tile([128, CB, KC_TOK], F16, tag="dch")
                nc.gpsimd.dma_start(out=d_ch[:, :, :],
                                    in_=d_re[:, :, k0:k0 + KC_TOK])
                x_ch = chunks.tile([128, KB, C], BF16, tag="xch")
                nc.sync.dma_start(out=x_ch[:, :, :],
                                  in_=x_re[:, KB * kc:KB * kc + KB, :])
                chunk_tiles[kc] = (d_ch, x_ch)

            # queue the per-core query slices on the sync engine ahead of
            # everything else there; lhs' is the first PE consumer
            dq_tiles = []
            for sc in range(NQT):
                dq_ch = chunks.tile([128, CB, QT], F16, tag="dq")
                nc.gpsimd.dma_start(out=dq_ch[:, :, :],
                                    in_=dq_re[:, :, sc * QT:(sc + 1) * QT])
                dq_tiles.append(dq_ch)

            if stage >= 2:
                issue_chunk(0)

            # w2 is only needed by the decoder at the very end
            w2_sb = consts.tile([128, CB, C], F32R, tag="w2")
            nc.gpsimd.dma_start(out=w2_sb[:, :, :],
                                in_=w2_t.rearrange("(cb p) co -> p cb co", p=128))

            # ---- lhs' = d_q^T M + u  (channel-major [co, q], fp16 out) ----
            lhs_sb = res.tile([128, CB, Q], F16)
            for sc in range(NQT):
                dq_ch = dq_tiles[sc]
                for co in range(CB):
                    ps = stp.tile([128, 512], F32, tag="st")
                    for ci in range(CB):
                        nc.tensor.matmul(
                            ps[:, :QT],
                            lhsT=m_sb[:, ci, co * 128:(co + 1) * 128],
                            rhs=dq_ch[:, ci, :],
                            start=(ci == 0), stop=(ci == CB - 1))
                    with nc.allow_low_precision(reason="fp16 lhs operand"):
                        nc.vector.tensor_scalar_add(
                            out=lhs_sb[:, co, sc * QT:(sc + 1) * QT],
                            in0=ps[:, :QT], scalar1=v_t["u"][:, co:co + 1])

            if stage == 1:
                lhs_f = res.tile([128, CB, Q], F32)
                nc.vector.tensor_copy(out=lhs_f[:, :, :], in_=lhs_sb[:, :, :])
                nc.sync.dma_start(out=y_re[:, :, :], in_=lhs_f[:, :, :])

            enh_acc = res.tile([128, CB, Q], F32R)      # [c%128, cb, q]
            # E-sum accumulators: chunks 0..nkc-2 in esum_acc (partition-
            # reduced early, overlapping the last chunk), last chunk separate
            esum_acc = res.tile([128, NQT * QT], F32)
            nc.vector.memset(esum_acc[:, :], 0.0)
            esum_last = res.tile([128, NQT * QT], F32)
            nc.vector.memset(esum_last[:, :], 0.0)
            den_a = consts.tile([128, NQT * QT], F32)
            nc.vector.memset(den_a[:, :], 0.0)

            # ---- main key loop: pure attention ----
            for kc in range(nkc if stage >= 2 else 0):
                d_ch, x_ch = chunk_tiles.pop(kc)
                if kc + 1 < nkc:
                    issue_chunk(kc + 1)
                if kc == nkc - 1 and nkc > 1:
                    # reduce chunks 0..nkc-2 on GpSimd while this chunk runs
                    nc.gpsimd.partition_all_reduce(
                        den_a[:, :], esum_acc[:, :], channels=128,
                        reduce_op=bass_isa.ReduceOp.add)
                e_all = eallp.tile([128, KB, NQT, QT], BF16, tag="eall")

                # S^T tiles + exp
                for kb in range(KB):
                    for qt in range(NQT):
                        st = stp.tile([128, 512], F32, tag="st")
                        for ci in range(CB):
                            nc.tensor.matmul(
                                st[:, :QT],
                                lhsT=d_ch[:, ci, kb * 128:(kb + 1) * 128],
                                rhs=lhs_sb[:, ci, qt * QT:(qt + 1) * QT],
                                start=(ci == 0), stop=(ci == CB - 1))
                        nc.scalar.activation(out=e_all[:, kb, qt, :],
                                             in_=st[:, :QT], func=AF.Exp,
                                             bias=shift_t[:, :], scale=1.0)

                # E-sum over key blocks: bf16 pairwise tree, then one
                # accumulate into the fp32 accumulator for this chunk group
                tgt = esum_last if (kc == nkc - 1 and nkc > 1) else esum_acc
                t01 = eallp.tile([128, NQT * QT], BF16, tag="t01")
                nc.vector.tensor_add(
                    out=t01[:, :],
                    in0=e_all[:, 0, :, :].rearrange("p a b -> p (a b)"),
                    in1=e_all[:, 1, :, :].rearrange("p a b -> p (a b)"))
                t23 = eallp.tile([128, NQT * QT], BF16, tag="t23")
                nc.vector.tensor_add(
                    out=t23[:, :],
                    in0=e_all[:, 2, :, :].rearrange("p a b -> p (a b)"),
                    in1=e_all[:, 3, :, :].rearrange("p a b -> p (a b)"))
                ts = eallp.tile([128, NQT * QT], BF16, tag="ts")
                nc.vector.tensor_add(out=ts[:, :], in0=t01[:, :], in1=t23[:, :])
                nc.vector.tensor_add(out=tgt[:, :], in0=tgt[:, :], in1=ts[:, :])

                # PV: enh_cm[c, q] += sum_k X[k, c] E[k, q]  (X stationary)
                # qt outer so the final chunk's qt0 completes early and the
                # epilogue (recip/normalize/W2) overlaps the qt1/qt2 PV work
                for qt in range(NQT):
                    for cb in range(CB):
                        pv = pvp.tile([128, 512], F32, tag="pv")
                        for kb in range(KB):
                            nc.tensor.matmul(
                                pv[:, :QT],
                                lhsT=x_ch[:, kb, cb * 128:(cb + 1) * 128],
                                rhs=e_all[:, kb, qt, :],
                                start=(kb == 0), stop=(kb == KB - 1))
                        with nc.allow_low_precision(reason="f32r is 32-bit"):
                            if kc == 0:
                                nc.vector.tensor_copy(
                                    out=enh_acc[:, cb, qt * QT:(qt + 1) * QT],
                                    in_=pv[:, :QT])
                            else:
                                nc.vector.tensor_add(
                                    out=enh_acc[:, cb, qt * QT:(qt + 1) * QT],
                                    in0=enh_acc[:, cb, qt * QT:(qt + 1) * QT],
                                    in1=pv[:, :QT])

            if stage == 2:
                nc.sync.dma_start(
                    out=y_re[:, :, :],
                    in_=enh_acc[:, :, :].rearrange("p a b -> p (a b)")
                    .rearrange("p (a n) -> p a n", a=CB))

            # ---- epilogue ----
            if stage >= 3:
                den_b = consts.tile([128, NQT * QT], F32)
                nc.gpsimd.partition_all_reduce(
                    den_b[:, :],
                    esum_last[:, :] if nkc > 1 else esum_acc[:, :],
                    channels=128, reduce_op=bass_isa.ReduceOp.add)
                nc.vector.tensor_add(out=den_b[:, :], in0=den_b[:, :],
                                     in1=den_a[:, :])
                rden = consts.tile([128, NQT * QT], F32R)

                # decoder on the UN-normalized accumulator: the den division
                # commutes past W2 and rides the PSUM drain, so the matmuls
                # depend only on enh_acc, not on the reciprocal chain. The
                # decoder bias b2 cancels exactly in BatchNorm and is dropped;
                # BN stats on the bias-free y are identical.
                y_sb = res.tile([128, CB, Q], F32)
                acc_y = consts.tile([128, CB, NQT], F32)
                acc_sq = consts.tile([128, CB, NQT], F32)
                for qt in range(NQT):
                    with nc.allow_low_precision(reason="f32r is 32-bit"):
                        nc.vector.reciprocal(
                            out=rden[:, qt * QT:(qt + 1) * QT],
                            in_=den_b[:, qt * QT:(qt + 1) * QT])
                    for co in range(CB):
                        # alternate PSUM pools: pv banks are free after the
                        # key loop, giving an 8-deep drain pipeline here
                        if (qt * CB + co) % 2 == 0:
                            ps = stp.tile([128, 512], F32, tag="st")
                        else:
                            ps = pvp.tile([128, 512], F32, tag="pv")
                        for ci in range(CB):
                            nc.tensor.matmul(
                                ps[:, :QT],
                                lhsT=w2_sb[:, ci, co * 128:(co + 1) * 128],
                                rhs=enh_acc[:, ci, qt * QT:(qt + 1) * QT],
                                start=(ci == 0), stop=(ci == CB - 1))
                        nc.vector.tensor_mul(
                            out=y_sb[:, co, qt * QT:(qt + 1) * QT],
                            in0=ps[:, :QT],
                            in1=rden[:, qt * QT:(qt + 1) * QT])
                        nc.vector.reduce_sum(
                            out=acc_y[:, co, qt:qt + 1],
                            in_=y_sb[:, co, qt * QT:(qt + 1) * QT],
                            axis=mybir.AxisListType.X)
                        jt = outp.tile([128, QT], F32, tag="junk")
                        nc.scalar.activation(
                            out=jt[:, :],
                            in_=y_sb[:, co, qt * QT:(qt + 1) * QT],
                            func=AF.Square,
                            accum_out=acc_sq[:, co, qt:qt + 1])

            if stage == 3:
                nc.sync.dma_start(out=y_re[:, :, :], in_=y_sb[:, :, :])
            if stage >= 4:
                sums = consts.tile([128, 2 * CB], F32)
                for cb in range(CB):
                    nc.vector.reduce_sum(out=sums[:, cb:cb + 1],
                                         in_=acc_y[:, cb, :],
                                         axis=mybir.AxisListType.X)
                    nc.vector.reduce_sum(out=sums[:, CB + cb:CB + cb + 1],
                                         in_=acc_sq[:, cb, :],
                                         axis=mybir.AxisListType.X)

                ar_in = dram.tile([128, 2 * CB], F32)
                ar_out = dram.tile([128, 2 * CB], F32)
                nc.sync.dma_start(out=ar_in[:], in_=sums[:, :])
                if use_collective:
                    nc.gpsimd.collective_compute(
                        "AllReduce", mybir.AluOpType.add,
                        replica_groups=[list(range(NCORES))],
                        ins=[ar_in.opt()], outs=[ar_out.opt()])
                else:
                    nc.gpsimd.dma_start(out=ar_out[:], in_=ar_in[:])
                gs = consts.tile([128, 2 * CB], F32)
                nc.sync.dma_start(out=gs[:, :], in_=ar_out[:])

                # mean/var -> scale/bias (one /N pass over both sums)
                mv = consts.tile([128, 2 * CB], F32)
                nc.vector.tensor_scalar_mul(out=mv[:, :], in0=gs[:, :],
                                            scalar1=1.0 / N)
                m2 = consts.tile([128, CB], F32)
                nc.vector.tensor_mul(out=m2[:, :], in0=mv[:, 0:CB],
                                     in1=mv[:, 0:CB])
                var_t = consts.tile([128, CB], F32)
                nc.vector.tensor_sub(out=var_t[:, :], in0=mv[:, CB:2 * CB],
                                     in1=m2[:, :])
                nc.vector.tensor_scalar_add(out=var_t[:, :], in0=var_t[:, :],
                                            scalar1=BN_EPS)
                sq = consts.tile([128, CB], F32)
                nc.scalar.sqrt(out=sq[:, :], in_=var_t[:, :])
                inv_t = consts.tile([128, CB], F32)
                nc.vector.reciprocal(out=inv_t[:, :], in_=sq[:, :])

                scale_t = consts.tile([128, CB], F32)
                nc.vector.tensor_mul(out=scale_t[:, :], in0=inv_t[:, :],
                                     in1=v_t["gamma"][:, :])
                bias2_t = consts.tile([128, CB], F32)
                nc.vector.tensor_mul(out=bias2_t[:, :], in0=mv[:, 0:CB],
                                     in1=scale_t[:, :])
                nc.vector.tensor_sub(out=bias2_t[:, :], in0=v_t["beta"][:, :],
                                     in1=bias2_t[:, :])

                # y = relu(scale * y + bias): blocks 0-1 on ACT, 2-3 on DVE
                # (parallel engines; 4 yo bufs so DMA never gates a ReLU)
                for cb in range(CB):
                    yo = outp.tile([128, Q], F32, tag="yo")
                    if cb < 2:
                        nc.scalar.activation(out=yo, in_=y_sb[:, cb, :],
                                             func=AF.Relu,
                                             scale=scale_t[:, cb:cb + 1],
                                             bias=bias2_t[:, cb:cb + 1])
                    else:
                        nc.vector.tensor_scalar(
                            out=yo[:, :], in0=y_sb[:, cb, :],
                            scalar1=scale_t[:, cb:cb + 1],
                            scalar2=bias2_t[:, cb:cb + 1],
                            op0=mybir.AluOpType.mult,
                            op1=mybir.AluOpType.add)
                        nc.vector.tensor_scalar_max(out=yo[:, :], in0=yo[:, :],
                                                    scalar1=0.0)
                    nc.sync.dma_start(out=y_re[:, cb, :], in_=yo)

    nc.finalize()
    return nc


def _prepare_in_maps(x, from_depth_estimation, w_rgb, b_rgb, w_lhs, b_lhs,
                     w_rhs, b_rhs, w_dec, b_dec, gamma, beta):
    f32, f64 = np.float32, np.float64
    d_cm = np.ascontiguousarray(
        np.asarray(from_depth_estimation, dtype=f32).transpose(1, 0, 2, 3)
        .reshape(C, N))
    d_cm16 = d_cm.astype(np.float16)
    x_tk = np.ascontiguousarray(
        np.asarray(x, dtype=f32).transpose(0, 2, 3, 1).reshape(N, C)
        .astype(ml_dtypes.bfloat16))
    wl = np.asarray(w_lhs, dtype=f64)
    wr = np.asarray(w_rhs, dtype=f64)
    wg = np.asarray(w_rgb, dtype=f64)
    wd = np.asarray(w_dec, dtype=f64)
    base = {
        "d_full": d_cm16, "x_tok": x_tk,
        "M_t": np.ascontiguousarray((wl.T @ wr).astype(np.float16)),
        "W2_t": np.ascontiguousarray((wd @ wg).T.astype(f32)),
        "u": (wr.T @ np.asarray(b_lhs, dtype=f64)).astype(f32),
        "b2": (wd @ np.asarray(b_rgb, dtype=f64)
               + np.asarray(b_dec, dtype=f64)).astype(f32),
        "gamma": np.asarray(gamma, dtype=f32),
        "beta": np.asarray(beta, dtype=f32),
    }
    in_maps = []
    for i in range(NCORES):
        m = dict(base)
        m["d_q"] = np.ascontiguousarray(d_cm16[:, i * Q:(i + 1) * Q])
        in_maps.append(m)
    return in_maps


def _assemble(results):
    out = np.empty((B, C, H, W), dtype=np.float32)
    rows = H // (NCORES // B)          # 24 rows of the image per core
    for i in range(NCORES):
        b, half = i // 2, i % 2
        out[b, :, half * rows:(half + 1) * rows, :] = (
            results[i]["y"].reshape(C, rows, W))
    return out


def kernel(x, from_depth_estimation, w_rgb, b_rgb, w_lhs, b_lhs, w_rhs, b_rhs,
           w_dec, b_dec, gamma, beta):
    global _nc_cache
    in_maps = _prepare_in_maps(x, from_depth_estimation, w_rgb, b_rgb, w_lhs,
                               b_lhs, w_rhs, b_rhs, w_dec, b_dec, gamma, beta)
    if _nc_cache is None:
        _nc_cache = _build()
    res = run_bass_kernel_spmd(_nc_cache, in_maps, list(range(NCORES)))
    return _assemble(res.results)
